# revision 1
# baseline (speedup 1.0000x reference)
"""AttentionPooling Trainium2 kernel.

Reference computation (per batch b):
    q   = q_emb[questions[b]]                      # (18, 128)
    qk  = (q @ x[b].T) / sqrt(128)                 # (18, 2048)
    attn= softmax(qk + log(mask))                  # masked softmax over s
    out = attn @ x[b]                              # (18, 128)

Strategy: data-parallel over batch across 8 cores (16 batches/core).
Per batch on-device:
  - load x[b] (2048,128) into SBUF as xn[p, c, d] with s = 16*p + c
    (16 chunks of 128 s-values on partitions), plus a ones column per
    chunk for the softmax denominator.
  - PE-transpose each 128x128 chunk -> xt[d, s] tile (matmul vs identity),
    PSUM->SBUF copies split between ScalarE/VectorE.
  - MM1: qkT[s_c, nq] = xt_c^T(weights) @ qT (host-gathered, pre-scaled)
  - exp on ScalarE straight out of PSUM (no max subtraction: |qk| <~ 6
    since inputs are N(0,1) and scaled by 1/sqrt(D); exp stays in fp32
    range), multiply by 0/1 mask (broadcast along nq).
  - MM2: out[nq, 0:129] accumulates attnT_c^T @ [x_c | 1] over chunks;
    column 128 is the softmax denominator.
  - normalize with reciprocal, DMA out.
"""

import math
from contextlib import ExitStack

import ml_dtypes
import numpy as np

import concourse.bass as bass
import concourse.tile as tile
from concourse import bacc, mybir
from concourse.bass_utils import run_bass_kernel_spmd
from concourse.masks import make_identity

B, S, D = 128, 2048, 128
NQ, QDIM = 18, 100
N_CORES = 8
BPC = B // N_CORES  # batches per core
C = 16              # s-chunks per batch (S = 128 * C), s = 16*p + c
CW = 130            # chunk width in xn tile: 128 data + 1 ones + 1 pad

_NC_CACHE: dict = {}


def build_nc(compute: str = "bf16", bpc: int = BPC, reps: int = 1,
             tile_t: str = "", tile_m1: str = "", stage: str = "full"):
    """Build the per-core bass program. compute in {'f32','bf16'}.

    reps > 1 wraps the whole batch loop in a hardware For_i that redoes the
    same work `reps` times (same data, same output) — benchmarking only.

    tile_t / tile_m1: column-tiling mode for the transposes / QK matmuls:
    "" (single full-width op), "2x64" (two 64-col tiles at col groups 0/64),
    "4x32" (four 32-col tiles — quadrant 3 hangs cayman, do not use).
    Splitting loads the stationary weights through parallel XBUSes.
    """

    def col_splits(mode):
        if mode == "2x64":
            return [(0, 64), (64, 64)]
        if mode == "4x32":
            return [(0, 32), (32, 32), (64, 32), (96, 32)]
        if mode == "3t":
            return [(0, 32), (32, 32), (64, 64)]
        return [(0, 128)]
    dt = mybir.dt.bfloat16 if compute == "bf16" else mybir.dt.float32
    f32 = mybir.dt.float32
    cast_load = compute == "bf16"

    nc = bacc.Bacc("TRN2", target_bir_lowering=False, debug=False)
    xs = nc.dram_tensor("xs", [bpc, S, D], f32, kind="ExternalInput").ap()
    qts = nc.dram_tensor("qts", [bpc, D, NQ], dt, kind="ExternalInput").ap()
    mks = nc.dram_tensor("mks", [bpc, 128, C], dt, kind="ExternalInput").ap()
    out = nc.dram_tensor("out", [bpc, NQ, D], f32, kind="ExternalOutput").ap()

    xr = xs.rearrange("b (p c) d -> b p c d", p=128)

    with tile.TileContext(nc) as tc:
        with ExitStack() as ctx:
            singles = ctx.enter_context(tc.tile_pool(name="singles", bufs=1))
            xn_pool = ctx.enter_context(tc.tile_pool(name="xn", bufs=3))
            xt_pool = ctx.enter_context(tc.tile_pool(name="xt", bufs=2))
            sm_pool = ctx.enter_context(tc.tile_pool(name="sm", bufs=3))
            e_pool = ctx.enter_context(tc.tile_pool(name="e", bufs=2))
            ob_pool = ctx.enter_context(tc.tile_pool(name="ob", bufs=3))
            ps_xt_pool = ctx.enter_context(
                tc.tile_pool(name="ps_xt", bufs=4, space="PSUM")
            )
            ps_qk_pool = ctx.enter_context(
                tc.tile_pool(name="ps_qk", bufs=2, space="PSUM")
            )
            ps_o_pool = ctx.enter_context(
                tc.tile_pool(name="ps_o", bufs=2, space="PSUM")
            )

            ident = singles.tile([128, 128], dt)
            make_identity(nc, ident[:])

            # all batches' qT and mask in one DMA each (tiny)
            qta = singles.tile([D, bpc, NQ], dt)
            nc.sync.dma_start(out=qta[:], in_=qts.rearrange("b p n -> p b n"))
            mka = singles.tile([128, bpc, C], dt)
            nc.sync.dma_start(out=mka[:], in_=mks.rearrange("b p c -> p b c"))

            def body(b):
                # ---- load x[b]: s=16p+c chunk layout, f32->dt cast in DMA
                xn = xn_pool.tile([128, C, CW], dt)
                eng = nc.gpsimd if cast_load else nc.sync
                eng.dma_start(out=xn[:, :, 0:D], in_=xr[b])
                nc.vector.memset(xn[:, :, D : D + 1], 1.0)

                qt = qta[:, b, :]
                mk = mka[:, b, :]

                if stage == "dma":
                    ob = ob_pool.tile([NQ, D], f32)
                    nc.vector.memset(ob[:], 0.0)
                    nc.sync.dma_start(out=out[b], in_=ob[:])
                    return

                # ---- transpose x chunks: xt[d, 16 chunks of 128 s]
                xt = xt_pool.tile([128, C * 128], dt)
                for g in range(4):
                    ps_xt = ps_xt_pool.tile([128, 512], dt)
                    for j in range(4):
                        c = 4 * g + j
                        dst_ps = ps_xt[:, j * 128 : (j + 1) * 128]
                        for off, w in col_splits(tile_t):
                            kw = {} if w == D else {"tile_position": (0, off)}
                            nc.tensor.transpose(
                                dst_ps[off : off + w, :],
                                xn[:, c, off : off + w],
                                ident[:],
                                **kw,
                            )
                    dst = xt[:, g * 512 : (g + 1) * 512]
                    if g % 2 == 0:
                        nc.scalar.copy(dst, ps_xt[:])
                    else:
                        nc.vector.tensor_copy(dst, ps_xt[:])

                if stage == "t":
                    ob = ob_pool.tile([NQ, D], f32)
                    nc.vector.memset(ob[:], 0.0)
                    nc.sync.dma_start(out=out[b], in_=ob[:])
                    return

                # ---- MM1: qkT[s, nq] per chunk (lhsT = xT_c weights)
                ps_qk = ps_qk_pool.tile([128, C, NQ], f32)
                for c in range(C):
                    for off, w in col_splits(tile_m1):
                        kw = {} if w == D else {"tile_position": (0, off)}
                        nc.tensor.matmul(
                            ps_qk[off : off + w, c, :],
                            lhsT=xt[:, c * 128 + off : c * 128 + off + w],
                            rhs=qt,
                            start=True,
                            stop=True,
                            **kw,
                        )

                if stage == "mm1":
                    ob = ob_pool.tile([NQ, D], f32)
                    nc.vector.memset(ob[:], 0.0)
                    nc.sync.dma_start(out=out[b], in_=ob[:])
                    return

                # ---- softmax numerator: exp, then mask (0/1) broadcast
                e = e_pool.tile([128, C, NQ], dt, tag="e")
                nc.scalar.activation(e[:], ps_qk[:], mybir.ActivationFunctionType.Exp)
                at = e_pool.tile([128, C, NQ], dt, tag="at")
                mk_b = mk.unsqueeze(2).broadcast_to([128, C, NQ])
                nc.vector.tensor_mul(at[:], e[:], mk_b)

                # ---- MM2: accumulate attnT_c^T @ [x_c | 1] over chunks
                ps_o = ps_o_pool.tile([NQ, D + 1], f32)
                for c in range(C):
                    nc.tensor.matmul(
                        ps_o[:],
                        lhsT=at[:, c, :],
                        rhs=xn[:, c, 0 : D + 1],
                        start=(c == 0),
                        stop=(c == C - 1),
                    )

                # ---- normalize and store
                r = sm_pool.tile([NQ, 1], f32, tag="r")
                nc.vector.reciprocal(r[:], ps_o[:, D : D + 1])
                ob = ob_pool.tile([NQ, D], f32)
                nc.scalar.activation(
                    ob[:],
                    ps_o[:, 0:D],
                    mybir.ActivationFunctionType.Copy,
                    scale=r[:],
                )
                nc.sync.dma_start(out=out[b], in_=ob[:])

            if reps > 1:
                with tc.For_i(0, reps, 1):
                    for b in range(bpc):
                        body(b)
            else:
                for b in range(bpc):
                    body(b)

    nc.compile()
    return nc


def _get_nc(compute: str = "bf16", bpc: int = BPC):
    key = (compute, bpc)
    if key not in _NC_CACHE:
        _NC_CACHE[key] = build_nc(compute, bpc)
    return _NC_CACHE[key]


def prep_inputs(x, q_emb, questions, mask, compute: str = "bf16"):
    """Host-side prep: gather+scale+transpose the tiny q table, reshape mask."""
    q_emb = np.asarray(q_emb, dtype=np.float32)
    questions = np.asarray(questions)
    mask = np.asarray(mask)
    np_dt = ml_dtypes.bfloat16 if compute == "bf16" else np.float32
    scale = 1.0 / math.sqrt(D)
    q = (q_emb * scale)[questions]                          # (B, NQ, D)
    qT = np.ascontiguousarray(q.transpose(0, 2, 1)).astype(np_dt)  # (B, D, NQ)
    mk = np.ascontiguousarray(mask.astype(np_dt).reshape(B, 128, C))  # s = 16p+c
    return qT, mk


def kernel(x, q_emb, questions, mask, compute: str = "bf16"):
    nc = _get_nc(compute)
    qT, mk = prep_inputs(x, q_emb, questions, mask, compute)
    x = np.ascontiguousarray(np.asarray(x), dtype=np.float32)

    in_maps = []
    for k in range(N_CORES):
        sl = slice(k * BPC, (k + 1) * BPC)
        in_maps.append({"xs": x[sl], "qts": qT[sl], "mks": mk[sl]})

    res = run_bass_kernel_spmd(nc, in_maps, core_ids=list(range(N_CORES)))
    outs = np.concatenate([res.results[k]["out"] for k in range(N_CORES)], axis=0)
    return np.ascontiguousarray(outs, dtype=np.float32)


if __name__ == "__main__":
    rng = np.random.default_rng(0)
    x = rng.standard_normal((B, S, D), dtype=np.float32)
    q_emb = rng.standard_normal((QDIM, D), dtype=np.float32)
    questions = rng.integers(0, QDIM, size=(B, NQ), dtype=np.int32)
    mask = rng.integers(0, 2, size=(B, S), dtype=np.int32)
    out = kernel(x, q_emb, questions, mask)
    print(out.shape, out.dtype)

